# revision 33
# baseline (speedup 1.0000x reference)
"""Multi-head attention forward for nn_AttentionStoreActivationPrune.

The reference's straight-through pattern ``sg(dense) + prune - sg(prune)``
is numerically ``dense`` in the forward pass, so the output equals a plain
multi-head attention forward.

Sharding: data-parallel over batch - 8 batch elements, one per NeuronCore.

Numerics (validated against a float64 reference, gate is rel<2e-2):
  - Q/K/V projections run as an exact-ish 3-term fp8-e4m3 DoubleRow
    decomposition  x@W = x8@w8 + x8@dw8 + dx8@w8  where w8 = fp8(16W),
    dw8 = fp8(16W - w8), x8 = fp8(x), dx8 = fp8(x - x8) are all
    host-prepared (same DMA bytes as bf16, 0.75x the PE cost, ~bf16
    accuracy).  The 16x weight scaling keeps the residuals out of fp8's
    subnormal floor; the scale is folded into the exp scale (1/256), the
    softmax denominator (ones column = 1 vs 16x ctx rows) and the final
    output copy (1/16).
  - Scores (QK^T) run as fp8 DoubleRow with on-device quantized q16/k16
    (zero second contraction tile via a step-sliced 7-slot layout).
  - ctx (P.V) and the out projection run in bf16.
  - End-to-end error ~1.4e-2 scale-relative on device (gate 2e-2).

Scheduling: engines execute in emission order, so projection / V /
out-projection matmul chains are emitted through a filler queue pulled
between attention steps - they fill the PE stall windows where attention
waits on Act (exp) or DVE (normalize).  One PSUM pool for the whole
program (tags: pq 1x2 banks for projection chains, ps 2x2 banks for
score pairs / warmup / out-proj, pc 2x1 banks for per-qc ctx), so there
are no pool-transition barriers.
"""

import numpy as np
import ml_dtypes

S, H, NH, HD, KO = 577, 768, 12, 64, 6
SPP = 580  # padded s
B = 8
SCH = [(0, 128), (128, 128), (256, 128), (384, 128), (512, 65)]
NQ = [(0, 290), (290, 290)]
NV = [(0, 512), (512, 256)]

F8 = ml_dtypes.float8_e4m3
BF16 = ml_dtypes.bfloat16

_CACHE = {}
LABELS = {}


def _lab(inst, label):
    try:
        LABELS[inst.ins.name] = label
    except Exception:
        pass
    return inst


def _build_nc(zero_bias, n_warm=12):
    import concourse.mybir as mybir
    import concourse.tile as tile
    from concourse import bacc

    f32 = mybir.dt.float32
    f8 = mybir.dt.float8e4
    bf = mybir.dt.bfloat16
    DR = mybir.MatmulPerfMode.DoubleRow
    EXP = mybir.ActivationFunctionType.Exp
    COPY = mybir.ActivationFunctionType.Copy
    ADD = mybir.AluOpType.add
    MULT = mybir.AluOpType.mult

    nc = bacc.Bacc("TRN2", target_bir_lowering=False, debug=False)

    xx8_d = nc.dram_tensor("xx8", [H, 2, SPP], f8, kind="ExternalInput")
    wqk_d = nc.dram_tensor("wqk", [128, KO, 4, KO, 128], f8,
                           kind="ExternalInput")
    wv2_d = nc.dram_tensor("wv2", [128, KO, 2, H], f8, kind="ExternalInput")
    wo_d = nc.dram_tensor("wo", [128, KO, KO, 128], bf, kind="ExternalInput")
    ident_d = nc.dram_tensor("ident", [128, 128], bf, kind="ExternalInput")
    if not zero_bias:
        bqk_d = nc.dram_tensor("bqk", [128, 2, KO], f32, kind="ExternalInput")
        bv_d = nc.dram_tensor("bv", [1, H], bf, kind="ExternalInput")
        bo_d = nc.dram_tensor("bo", [128, KO], f32, kind="ExternalInput")
    out_d = nc.dram_tensor("out", [H, SPP], bf, kind="ExternalOutput")

    with tile.TileContext(nc) as tc:
        with tc.tile_pool(name="consts", bufs=1) as consts, \
             tc.tile_pool(name="bigs", bufs=1) as bigs, \
             tc.tile_pool(name="epool", bufs=8) as epool, \
             tc.tile_pool(name="mid", bufs=3) as mid, \
             tc.tile_pool(name="outs", bufs=6) as outsp:

            warm = consts.tile([128, 512], bf, tag="warm")
            nc.gpsimd.memset(warm, 0.0)
            if not zero_bias:
                onesrow = consts.tile([1, 128], bf, tag="onesrow")
                nc.vector.memset(onesrow, 1.0)
                bqk_t = consts.tile([128, 2, KO], f32, tag="bqk")
                nc.sync.dma_start(out=bqk_t, in_=bqk_d[:])
                bv_t = consts.tile([1, H], bf, tag="bv")
                nc.sync.dma_start(out=bv_t, in_=bv_d[:])
                bo_t = consts.tile([128, KO], f32, tag="bo")
                nc.sync.dma_start(out=bo_t, in_=bo_d[:])

            xx8t = bigs.tile([128, KO, 2, SPP], f8, tag="xx8t")
            x8t = xx8t[:, :, 0]
            dx8t = xx8t[:, :, 1]
            wqk = bigs.tile([128, KO, 4, KO, 128], f8, tag="wqk")
            wv2 = bigs.tile([128, KO, 2, H], f8, tag="wv2")
            wo16 = bigs.tile([128, KO, KO, 128], bf, tag="wo16")
            QTa = bigs.tile([128, 7, SPP], f8, tag="QTa")
            KTa = bigs.tile([128, 7, SPP], f8, tag="KTa")
            Vaug = [bigs.tile([128, NH, 66], bf, tag=f"vaug{i}",
                              name=f"vaug{i}")
                    for i in range(len(SCH))]
            CTXU = bigs.tile([128, KO, SPP], bf, tag="CTXU")
            outAacc = bigs.tile([128, KO, SPP], bf, tag="outAacc")
            ident = bigs.tile([128, 128], bf, tag="ident")

            # DoubleRow zero slots + denominator ones columns
            nc.gpsimd.memset(QTa[:, KO, :], 0.0)
            nc.gpsimd.memset(KTa[:, KO, :], 0.0)
            for sc in range(len(SCH)):
                nc.gpsimd.memset(Vaug[sc][:, :, HD:HD + 1], 1.0)

            # ---- input DMAs in consumption order; dx8 rides a second
            # queue (DVE) so its transfers interleave with the sync queue ----
            x8_src = x8_d.rearrange("(ko ki) s -> ki ko s", ki=128)
            dx8_src = dx8_d.rearrange("(ko ki) s -> ki ko s", ki=128)
            nc.sync.dma_start(out=wqk[:, 0, 0:2], in_=wqk_d[:, 0, 0:2])
            for k0 in range(0, KO, 2):
                nc.sync.dma_start(out=x8t[:, k0:k0 + 2, :],
                                  in_=x8_src[:, k0:k0 + 2, :])
            nc.sync.dma_start(out=wqk[:, 0, 2:4], in_=wqk_d[:, 0, 2:4])
            for k0 in range(0, KO, 2):
                nc.sync.dma_start(out=dx8t[:, k0:k0 + 2, :],
                                  in_=dx8_src[:, k0:k0 + 2, :])
            nc.scalar.dma_start(out=ident, in_=ident_d[:])
            for k0 in range(0, KO, 2):
                nc.sync.dma_start(out=wv2[:, k0:k0 + 2],
                                  in_=wv2_d[:, k0:k0 + 2])
            nc.sync.dma_start(out=wqk[:, 1], in_=wqk_d[:, 1])
            nc.sync.dma_start(out=wqk[:, 2:4], in_=wqk_d[:, 2:4])
            nc.sync.dma_start(out=wqk[:, 4:6], in_=wqk_d[:, 4:6])
            nc.sync.dma_start(out=wo16[:, 0:3], in_=wo_d[:, 0:3])
            nc.sync.dma_start(out=wo16[:, 3:6], in_=wo_d[:, 3:6])

            with tc.tile_pool(name="pp", bufs=1, space="PSUM") as pp:

                def tile_pq(name):
                    return pp.tile([128, 512], f32, tag="pq", bufs=2,
                                   name=name)

                def tile_ps(name):
                    return pp.tile([128, 1024], f32, tag="ps", bufs=2,
                                   name=name)

                def tile_pc(name):
                    return pp.tile([65, 290], f32, tag="pc", bufs=2,
                                   name=name)

                # PE warm-up fodder: keeps the PE p-state ramp going while
                # the first DMAs land
                for wi in range(n_warm):
                    pw = tile_ps(f"warm{wi}")
                    nc.tensor.matmul(pw[:, 0:256], warm[:, 0:128],
                                     warm[:, 0:256], start=True, stop=True)

                def gen_proj_qk(koh, pause=0):
                    # q16/k16 of head pair koh: 3-term fp8 DR chains, then
                    # fp8 quantize into QTa/KTa slot koh
                    for _ in range(pause):
                        yield
                    for which, (dst, wbase) in enumerate(((QTa, 0),
                                                          (KTa, 1))):
                        for qc, (n0, nn) in enumerate(NQ):
                            pq = tile_pq(f"pq{koh}_{which}{qc}")
                            terms = ((x8t, wbase), (x8t, wbase + 2),
                                     (dx8t, wbase))
                            for t, (xs, ws) in enumerate(terms):
                                for kp in range(3):
                                    _lab(nc.tensor.matmul(
                                        pq[:, 0:nn],
                                        wqk[:, koh, ws, 2 * kp:2 * kp + 2, :],
                                        xs[:, 2 * kp:2 * kp + 2, n0:n0 + nn],
                                        start=(t == 0 and kp == 0),
                                        stop=(t == 2 and kp == 2),
                                        perf_mode=DR,
                                    ), f"projqk{koh}.{which}.q{qc}.t{t}k{kp}")
                                    yield
                            dstv = dst[:, koh, n0:n0 + nn]
                            if zero_bias:
                                nc.vector.tensor_copy(out=dstv,
                                                      in_=pq[:, 0:nn])
                            else:
                                nc.vector.tensor_scalar(
                                    dstv, pq[:, 0:nn],
                                    bqk_t[:, which, koh:koh + 1], None, ADD)
                            for _ in range(2):
                                yield

                def emit_proj_v(sc):
                    if True:
                        s0, sz = SCH[sc]
                        for vc, (n0, nn) in enumerate(NV):
                            pv = tile_pq(f"pv{sc}_{vc}")
                            terms = ((x8t, 0), (x8t, 1), (dx8t, 0))
                            for t, (xs, ws) in enumerate(terms):
                                for kp in range(3):
                                    _lab(nc.tensor.matmul(
                                        pv[0:sz, 0:nn],
                                        xs[:, 2 * kp:2 * kp + 2, s0:s0 + sz],
                                        wv2[:, 2 * kp:2 * kp + 2, ws,
                                            n0:n0 + nn],
                                        start=(t == 0 and kp == 0),
                                        stop=(t == 2 and kp == 2
                                              and zero_bias),
                                        perf_mode=DR,
                                    ), f"projv{sc}.{vc}.t{t}k{kp}")
                            if not zero_bias:
                                nc.tensor.matmul(
                                    pv[0:sz, 0:nn],
                                    onesrow[0:1, 0:sz],
                                    bv_t[0:1, n0:n0 + nn],
                                    start=False, stop=True,
                                )
                            h0 = n0 // HD
                            nc.vector.tensor_copy(
                                out=Vaug[sc][0:sz, h0:h0 + nn // HD, 0:HD],
                                in_=pv[0:sz, 0:nn].rearrange(
                                    "p (h d) -> p h d", d=HD),
                            )

                def gen_out_a():
                    for oc in range(KO):
                        for qc, (n0, nn) in enumerate(NQ):
                            poa = tile_pq(f"poa{oc}_{qc}")
                            for ko in range(3):
                                _lab(nc.tensor.matmul(
                                    poa[:, 0:nn],
                                    wo16[:, oc, ko, :],
                                    CTXU[:, ko, n0:n0 + nn],
                                    start=(ko == 0), stop=(ko == 2),
                                ), f"outA{oc}.q{qc}.k{ko}")
                                yield
                            nc.vector.tensor_copy(
                                out=outAacc[:, oc, n0:n0 + nn],
                                in_=poa[:, 0:nn])
                            yield

                def gen_out_proj():
                    for oc in range(KO):
                        po = tile_ps(f"po{oc}")
                        for qc, (n0, nn) in enumerate(NQ):
                            _lab(nc.tensor.matmul(
                                po[:, qc * 512:qc * 512 + nn],
                                ident[:, :],
                                outAacc[:, oc, n0:n0 + nn],
                                start=True, stop=False,
                            ), f"outB{oc}.q{qc}.unpark")
                            yield
                            for ko in range(3, KO):
                                _lab(nc.tensor.matmul(
                                    po[:, qc * 512:qc * 512 + nn],
                                    wo16[:, oc, ko, :],
                                    CTXU[:, ko, n0:n0 + nn],
                                    start=False, stop=(ko == KO - 1),
                                ), f"outB{oc}.q{qc}.k{ko}")
                                yield
                        OT = outsp.tile([128, SPP], bf, tag="ot",
                                        name=f"ot{oc}")
                        for qc, (n0, nn) in enumerate(NQ):
                            pot, off = sl[qc]
                            otv = OT[:, n0:n0 + nn]
                            pov = pot[:, off:off + nn]
                            if qc == 0:
                                if zero_bias:
                                    nc.vector.tensor_scalar(
                                        otv, pov, 1.0 / 16.0, None, MULT)
                                else:
                                    nc.vector.tensor_scalar(
                                        otv, pov, 1.0 / 16.0,
                                        bo_t[:, oc:oc + 1], MULT, ADD)
                            else:
                                bias = (0.0 if zero_bias
                                        else bo_t[:, oc:oc + 1])
                                nc.scalar.activation(out=otv, in_=pov,
                                                     func=COPY,
                                                     scale=1.0 / 16.0,
                                                     bias=bias)
                            yield
                        eng = nc.sync if oc % 2 == 0 else nc.scalar
                        eng.dma_start(out=out_d[oc * 128:(oc + 1) * 128, :],
                                      in_=OT)
                        yield

                fillers = []
                background = []

                def fill(n):
                    while n > 0:
                        q = fillers if fillers else background
                        if not q:
                            return
                        try:
                            next(q[0])
                            n -= 1
                        except StopIteration:
                            q.pop(0)

                def attend(h, ctx_lag=2):
                    koh, half = divmod(h, 2)
                    kb = half * HD
                    ss = slice(koh, 7, KO - koh)  # {koh, 6}: data + zeros
                    pcs = [tile_pc(f"pc{h}_{qc}") for qc in range(2)]
                    Es = []

                    def ctx_step(sc):
                        s0, sz = SCH[sc]
                        for qc in range(2):
                            _lab(nc.tensor.matmul(
                                pcs[qc][0:65, :],
                                Vaug[sc][0:sz, h, 0:HD + 1],
                                Es[sc][0:sz, qc, :],
                                start=(sc == 0), stop=(sc == len(SCH) - 1),
                            ), f"ctx{h}.s{sc}.q{qc}")
                            fill(1)

                    for sc, (s0, sz) in enumerate(SCH):
                        if h == 0:
                            emit_proj_v(sc)
                        ps = tile_ps(f"ps{h}_{sc}")
                        for qc, (n0, nn) in enumerate(NQ):
                            _lab(nc.tensor.matmul(
                                ps[0:sz, qc * 512:qc * 512 + nn],
                                KTa[kb:kb + HD, ss, s0:s0 + sz],
                                QTa[kb:kb + HD, ss, n0:n0 + nn],
                                start=True, stop=True,
                                perf_mode=DR,
                            ), f"score{h}.s{sc}.q{qc}")
                        E = epool.tile([128, 2, 290], bf, tag="e",
                                       name=f"e{h}_{sc}")
                        psv = ps.rearrange("p (b c) -> p b c",
                                           c=512)[0:sz, :, 0:290]
                        if h == 0:
                            for qc in range(2):
                                _lab(nc.scalar.activation(
                                    out=E[0:sz, qc:qc + 1],
                                    in_=psv[:, qc:qc + 1],
                                    func=EXP, scale=0.125 / 256.0,
                                ), f"exp{h}.s{sc}.q{qc}")
                        else:
                            _lab(nc.scalar.activation(
                                out=E[0:sz], in_=psv,
                                func=EXP, scale=0.125 / 256.0,
                            ), f"exp{h}.s{sc}")
                        Es.append(E)
                        fill(5 if sc < 2 else 3)
                        if sc >= ctx_lag:
                            ctx_step(sc - ctx_lag)
                    for sc in range(len(SCH) - ctx_lag, len(SCH)):
                        ctx_step(sc)
                    for qc, (n0, nn) in enumerate(NQ):
                        recipS = mid.tile([1, 290], f32, tag="recf",
                                          name=f"recf{h}_{qc}")
                        nc.vector.reciprocal(out=recipS,
                                             in_=pcs[qc][64:65, :])
                        bc = mid.tile([64, 290], f32, tag="bcast",
                                      name=f"bcast{h}_{qc}")
                        nc.gpsimd.partition_broadcast(bc, recipS)
                        nc.vector.tensor_mul(
                            out=CTXU[kb:kb + HD, koh, n0:n0 + nn],
                            in0=pcs[qc][0:HD, :],
                            in1=bc,
                        )

                # head-pair 0's four projection chains emit round-robin by
                # (term, kpair) so PE processes steps in DMA-arrival order
                # pair 0: all four chains fully round-robin across both
                # PSUM rings (qc0 on pq, qc1 on ps), quantize copies split
                # DVE / Act so the first exp starts as early as possible
                pq0 = {(0, 0): tile_pq("pq0_00"), (1, 0): tile_pq("pq0_10"),
                       (0, 1): tile_ps("pq0_01"), (1, 1): tile_ps("pq0_11")}
                for t in range(3):
                    for kp in range(3):
                        for (w, wbase) in ((0, 0), (1, 1)):
                            xs = dx8t if t == 2 else x8t
                            ws = wbase + 2 if t == 1 else wbase
                            for qc, (n0, nn) in enumerate(NQ):
                                _lab(nc.tensor.matmul(
                                    pq0[(w, qc)][:, 0:nn],
                                    wqk[:, 0, ws, 2 * kp:2 * kp + 2, :],
                                    xs[:, 2 * kp:2 * kp + 2, n0:n0 + nn],
                                    start=(t == 0 and kp == 0),
                                    stop=(t == 2 and kp == 2),
                                    perf_mode=DR,
                                ), f"projqk0.{w}.q{qc}.t{t}k{kp}")
                for (w, qc), pq in pq0.items():
                    dst = (QTa, KTa)[w]
                    n0, nn = NQ[qc]
                    if zero_bias:
                        if qc == 0:
                            nc.vector.tensor_copy(out=dst[:, 0, n0:n0 + nn],
                                                  in_=pq[:, 0:nn])
                        else:
                            nc.scalar.activation(out=dst[:, 0, n0:n0 + nn],
                                                 in_=pq[:, 0:nn], func=COPY)
                    else:
                        nc.vector.tensor_scalar(
                            dst[:, 0, n0:n0 + nn], pq[:, 0:nn],
                            bqk_t[:, w, 0:1], None, ADD)
                for koh in range(KO):
                    attend(2 * koh, ctx_lag=3 if koh == 0 else CTX_LAG)
                    if koh < KO - 1:
                        fillers.append(gen_proj_qk(koh + 1, pause=4))
                    if koh == 2:
                        background.append(gen_out_a())
                    attend(2 * koh + 1, ctx_lag=CTX_LAG)
                # drain leftover fillers, then the out projection
                while fillers or background:
                    fill(1000)
                for _ in gen_out_proj():
                    pass

    nc.finalize()
    return nc


def _prep_weights(Wq, Wk, Wv, Wo):
    """Host-side weight quantization + layout packing (pure layout/precision
    prep; all math stays on device)."""
    def split8(w16):
        w8 = w16.astype(F8)
        dw8 = (w16 - w8.astype(np.float32)).astype(F8)
        return w8, dw8

    q8, dq8 = split8(16.0 * Wq.astype(np.float32))
    k8, dk8 = split8(16.0 * Wk.astype(np.float32))
    v8, dv8 = split8(16.0 * Wv.astype(np.float32))

    # wqk [ki, koh, slot(q8,dq8,k8,dk8), ko, 128]
    def qk_layout(a):
        # a [in=768, out=768] -> [ki, koh, ko, c]
        return np.ascontiguousarray(
            a.reshape(KO, 128, KO, 128).transpose(1, 2, 0, 3))

    wqk = np.stack([qk_layout(a) for a in (q8, k8, dq8, dk8)],
                   axis=2)  # [ki, koh, 4, ko, c]

    # wv2 [ki, ko, slot(v8,dv8), H]
    def v_layout(a):
        return np.ascontiguousarray(a.reshape(KO, 128, H).transpose(1, 0, 2))

    wv2 = np.stack([v_layout(v8), v_layout(dv8)], axis=2)
    # wo16 [ki, oc, ko, c] bf16 (unscaled)
    wo16 = np.ascontiguousarray(
        Wo.astype(np.float32).reshape(KO, 128, KO, 128).transpose(1, 2, 0, 3)
    ).astype(BF16)
    return (np.ascontiguousarray(wqk), np.ascontiguousarray(wv2), wo16)


def kernel(hidden_states, Wq, bq, Wk, bk, Wv, bv, Wo, bo):
    from concourse.bass_utils import run_bass_kernel_spmd

    zero_bias = not (np.any(bq) or np.any(bk) or np.any(bv) or np.any(bo))
    key = ("nc", zero_bias)
    if key not in _CACHE:
        _CACHE[key] = _build_nc(zero_bias)
    nc = _CACHE[key]

    wqk, wv2, wo16 = _prep_weights(Wq, Wk, Wv, Wo)
    common = {"wqk": wqk, "wv2": wv2, "wo": wo16,
              "ident": np.eye(128, dtype=BF16)}
    if not zero_bias:
        bq16 = (16.0 * bq).astype(np.float32).reshape(KO, 128).T
        bk16 = (16.0 * bk).astype(np.float32).reshape(KO, 128).T
        common["bqk"] = np.ascontiguousarray(
            np.stack([bq16, bk16], axis=1))  # [128, 2, KO]
        common["bv"] = (16.0 * bv).astype(np.float32).reshape(1, H).astype(BF16)
        common["bo"] = np.ascontiguousarray(
            bo.astype(np.float32).reshape(KO, 128).T)

    hs = np.ascontiguousarray(hidden_states, np.float32)
    xts = np.zeros((B, H, SPP), np.float32)
    xts[:, :, :S] = hs.transpose(0, 2, 1)
    x8 = xts.astype(F8)
    dx8 = (xts - x8.astype(np.float32)).astype(F8)
    xx8 = np.ascontiguousarray(np.stack([x8, dx8], axis=2))  # [B, H, 2, SPP]

    in_maps = [dict(common, xx8=xx8[b]) for b in range(B)]
    res = run_bass_kernel_spmd(nc, in_maps, core_ids=list(range(B)))
    out = np.stack(
        [np.asarray(r["out"])[:, :S].astype(np.float32).T
         for r in res.results], axis=0)
    return np.ascontiguousarray(out)


if __name__ == "__main__":
    rng = np.random.default_rng(0)
    inputs = {
        "hidden_states": rng.standard_normal((B, S, H)).astype(np.float32),
        "Wq": (rng.standard_normal((H, H)) * 0.02).astype(np.float32),
        "bq": np.zeros(H, np.float32),
        "Wk": (rng.standard_normal((H, H)) * 0.02).astype(np.float32),
        "bk": np.zeros(H, np.float32),
        "Wv": (rng.standard_normal((H, H)) * 0.02).astype(np.float32),
        "bv": np.zeros(H, np.float32),
        "Wo": (rng.standard_normal((H, H)) * 0.02).astype(np.float32),
        "bo": np.zeros(H, np.float32),
    }
    got = kernel(**inputs)
    print("kernel output:", got.shape, got.dtype)
